# revision 26
# baseline (speedup 1.0000x reference)
"""Causal multi-head attention kernel for TRN2 (8 NeuronCores, SPMD).

Problem: x[2,2048,1024], per-head W_qkv[16,1024,192], W_out[16,64,1024].
  qkv = einsum('bsd,ndh->bnsh', x, W_qkv); causal softmax attention per head;
  out.reshape(B,-1,S); einsum('bds,nhd->bsd', out, W_out).

The final einsum does NOT contract d (it appears in both operands and the
output), so it reduces to a per-column scale by W_sum[d] = sum_{n,h} W_out —
pure data movement, done on the host.  The device computes the attention.

Sharding: 2 batches x 16 heads = 32 jobs; core c handles batch c//4 and the
4 heads [4*(c%4), 4*(c%4)+4), as 2 head-pairs packed into 128 partitions.

Device per core:
  - QKV projection in fp8e4 DoubleRow (x fp8; W fp8 pre-scaled x16 past the
    e4m3 subnormal floor): two 128-row d-subtiles per matmul pass.
  - Q^T/K^T evacuated fp16 (scores fp16: x16 scale cancels via exp scale).
  - V^T via fp16 PE transpose into [k, Va|1|0.. , Vb|1|0..] fp8 tiles
    (power-of-2 128-wide per head for fp8 LDWEIGHTS ISA rules); the ones
    column makes the AV matmul also produce the softmax denominator.
  - scores: per k-tile, 2 fp16 matmuls (heads packed via tile_position)
    into one [128,1024] fp32 PSUM tile; ONE exp per k-tile on ScalarE with
    scale=1/(8*256), bias=-2 (keeps exp inside the e4m3 +-240 range;
    cancels in normalization), writing fp8 P directly.
  - causal mask: only the 128-col diagonal strip of a crossing tile needs
    element masking (one shared triangle for every rel) — strided fp8
    tensor_mul on the otherwise idle GpSimd engine.
  - AV in fp8: non-crossing k-tiles accumulate two-at-a-time with DoubleRow
    ([128,2,128] stationary); crossing tiles as plain fp8 matmuls with
    causal column trim.  AV issue lags scores by 2 units so the PE never
    stalls on the ScalarE exp / GpSimd mask chain.
  - O'^T: rows 0..63 attention output (x16 from W scaling), row 64
    denominator; staged fp16 and DMA'd out per (pair, q-block).
  - PE warmup matmuls cover the initial DMA wait (p-state ramp).
Host epilogue: normalize, un-scale, reshape, scale by W_sum; rows < 256
(numerically degenerate for fp8: tiny softmax support exposes single-element
V/P quantization directly) are recomputed exactly — 1.5% of attention flops.
"""

import numpy as np
import ml_dtypes

import concourse.bass as bass
import concourse.mybir as mybir
from concourse.tile import TileContext
from concourse.bass_utils import run_bass_kernel_spmd

F32 = mybir.dt.float32
F16 = mybir.dt.float16
F8 = mybir.dt.float8e4
NP8 = ml_dtypes.float8_e4m3
DR = mybir.MatmulPerfMode.DoubleRow

B, S, D, NH, HD = 2, 2048, 1024, 16, 64  # batch, seq, model, heads, head_dim
NCORES = 8
HPC = 4  # heads per core
NPAIR = 2  # head pairs per core
DT = D // 128  # 8 D-tiles
NQB = S // 512  # 4 q blocks
NKT = S // 128  # 16 k tiles
WSCALE = 16.0  # host pre-scale on W_qkv to lift fp8 subnormals
EXP_SCALE = (1.0 / np.sqrt(HD)) / (WSCALE * WSCALE)
EXP_BIAS = -2.0
PATCH_ROWS = 256
# non-crossing pair indices whose exp runs on DVE (Schraudolph) per q-block
SCHRAUD = {2: (1,), 3: (1, 3, 5)}


def _split_excess_waits(nc, limit=1):
    """This walrus build rejects >1 sync-wait per instruction; hoist extra
    waits onto preceding same-engine no-ops (identical blocking semantics)."""
    cnt = 0
    for fn in nc.m.functions:
        for blk in fn.blocks:
            out = []
            for inst in blk.instructions:
                si = inst.sync_info
                if si is not None and si.on_wait and len(si.on_wait) > limit:
                    waits = list(si.on_wait)
                    excess, keep = waits[:-limit], waits[-limit:]
                    for i in range(0, len(excess), limit):
                        nop = mybir.InstNoOp(
                            name=f"wsplit_{cnt}", ins=[], outs=[], engine=inst.engine
                        )
                        cnt += 1
                        nop.sync_info = mybir.SyncInfo(
                            on_wait=excess[i : i + limit], on_update=[]
                        )
                        out.append(nop)
                    inst.sync_info = mybir.SyncInfo(
                        on_wait=keep, on_update=list(si.on_update or [])
                    )
                out.append(inst)
            blk.instructions = out
    return cnt


def build_nc(split_waits=True):
    nc = bass.Bass()
    xT = nc.declare_dram_parameter("xT", [D, S], F8, isOutput=False)
    w = nc.declare_dram_parameter("w", [NPAIR, 3, DT, 128, 128], F8, isOutput=False)
    tri = nc.declare_dram_parameter("tri", [128, 256], F8, isOutput=False)
    trizt = nc.declare_dram_parameter("trizt", [128, 512], F8, isOutput=False)
    ident = nc.declare_dram_parameter("ident", [128, 128], F16, isOutput=False)
    out = nc.declare_dram_parameter("out", [65, HPC * S], F16, isOutput=True)

    with TileContext(nc) as tc:
        with (
            tc.tile_pool(name="persist", bufs=1) as pp,
            tc.tile_pool(name="psum", bufs=3, space="PSUM") as ps,
            tc.tile_pool(name="work", bufs=2) as pc,
        ):
            # ---- persistent SBUF tensors (Q^T and K^T head-packed [2H, S])
            qt2 = [
                pp.tile([128, S], F16, tag=f"qt{p}", name=f"qtt{p}")
                for p in range(NPAIR)
            ]
            kt2 = [
                pp.tile([128, S], F16, tag=f"kt{p}", name=f"ktt{p}")
                for p in range(NPAIR)
            ]
            v2e = [
                pp.tile([128, NKT, 256], F8, tag=f"v2e{p}", name=f"v2e{p}")
                for p in range(NPAIR)
            ]
            tri_sb = pp.tile([128, 256], F8, tag="tri", name="tri_sb")
            trizt_sb = pp.tile([128, 512], F8, tag="trizt", name="trizt_sb")
            ident_sb = pp.tile([128, 128], F16, tag="ident", name="ident_sb")
            xt_sb = pp.tile([128, DT, S], F8, tag="xt", name="xt_sb")
            w_sb = pp.tile([128, NPAIR * 3 * DT, 128], F8, tag="w", name="w_sb")
            vt = [
                pp.tile([128, S], F16, tag=f"vt{p}", name=f"vt{p}")
                for p in range(NPAIR)
            ]
            vtT = [
                pp.tile([128, NKT, 128], F16, tag=f"vtT{p}", name=f"vtT{p}")
                for p in range(NPAIR)
            ]
            warm_sb = pp.tile([128, 640], F16, tag="warm", name="warm_sb")
            bias_sb = pp.tile([128, 1], F32, tag="bias", name="bias_sb")

            # engine-local constants / warmup source (no DMA dependency)
            nc.vector.memset(warm_sb[:], 0.0)
            nc.vector.memset(bias_sb[:], EXP_BIAS)
            for p in range(NPAIR):
                nc.vector.memset(v2e[p][:, :, 64], 1.0)
                nc.vector.memset(v2e[p][:, :, 192], 1.0)
                nc.vector.memset(v2e[p][:, :, 65:128], 0.0)
                nc.vector.memset(v2e[p][:, :, 193:256], 0.0)

            # PE warmup: keep the PE busy during the input DMA so the
            # p-state ramp completes before the first real matmul.
            wt = ps.tile([128, 512], F32, tag="mm", name="warmup")
            for _ in range(10):
                nc.tensor.matmul(
                    wt[:], warm_sb[:, 0:128], warm_sb[:, 128:640],
                    start=True, stop=True,
                )

            # DMA order = consumption order: pair-0 Q weights + first xT half
            # gate the first projection matmuls.
            w_v = w.rearrange("a t d k m -> k (a t d) m")
            xt_v = xT.rearrange("(dt p) s -> p dt s", p=128)
            nc.sync.dma_start(out=xt_sb[:, :, 0:512], in_=xt_v[:, :, 0:512])
            nc.sync.dma_start(out=w_sb[:, 0:DT, :], in_=w_v[:, 0:DT, :])
            nc.sync.dma_start(
                out=w_sb[:, DT : 3 * DT, :], in_=w_v[:, DT : 3 * DT, :]
            )
            nc.sync.dma_start(
                out=w_sb[:, 3 * DT : 6 * DT, :], in_=w_v[:, 3 * DT : 6 * DT, :]
            )
            nc.scalar.dma_start(out=xt_sb[:, :, 512:1024], in_=xt_v[:, :, 512:1024])
            nc.scalar.dma_start(out=xt_sb[:, :, 1024:1536], in_=xt_v[:, :, 1024:1536])
            nc.scalar.dma_start(out=xt_sb[:, :, 1536:2048], in_=xt_v[:, :, 1536:2048])
            nc.gpsimd.dma_start(out=ident_sb[:], in_=ident[:])
            nc.gpsimd.dma_start(out=tri_sb[:], in_=tri[:])
            nc.gpsimd.dma_start(out=trizt_sb[:], in_=trizt[:])

            def proj_step_gen(qcp, p, t):
                """Project one (pair, qkv-type) for q-columns
                [qcp*1024, (qcp+1)*1024) with fp8 DoubleRow (2 d-subtiles per
                pass); transpose V k-tiles of the chunk.  Yields between
                pieces so callers can interleave it into attention blocks."""
                acc2 = ps.tile([128, 1024], F32, tag="mm", name="acc2")
                for dp in range(DT // 2):
                    if dp:
                        yield
                    base = (p * 3 + t) * DT + 2 * dp
                    wsl = w_sb[:, base : base + 2, :]
                    for h in range(2):
                        qc = 2 * qcp + h
                        nc.tensor.matmul(
                            acc2[:, h * 512 : (h + 1) * 512],
                            wsl,
                            xt_sb[:, 2 * dp : 2 * dp + 2, qc * 512 : (qc + 1) * 512],
                            start=(dp == 0),
                            stop=(dp == DT // 2 - 1),
                            perf_mode=DR,
                        )
                yield
                csl = slice(qcp * 1024, (qcp + 1) * 1024)
                if t == 0:
                    nc.vector.tensor_copy(qt2[p][:, csl], acc2[:])
                elif t == 1:
                    nc.vector.tensor_copy(kt2[p][:, csl], acc2[:])
                else:  # V -> fp16; transpose the chunk off-PE via DMA xbar
                    nc.vector.tensor_copy(vt[p][:, csl], acc2[:])
                    nc.sync.dma_start_transpose(
                        vtT[p][:, 8 * qcp : 8 * qcp + 8, :], vt[p][:, csl]
                    )
                    for k in range(8 * qcp, 8 * qcp + 8):
                        if k % 4 == 2:
                            yield
                        # one strided fp16->fp8 copy into the padded layout
                        nc.gpsimd.tensor_copy(
                            v2e[p][:, k, :].rearrange("p (t c) -> p t c", t=2)[
                                :, :, 0:64
                            ],
                            vtT[p][:, k, :].rearrange("p (t c) -> p t c", t=2),
                        )

            def proj_step(qcp, p, t):
                for _ in proj_step_gen(qcp, p, t):
                    pass

            def attention(p, qb, filler=None):
                nk = 4 * (qb + 1)  # causal: k tiles 0..nk-1
                oa = ps.tile([128, 512], F32, tag="acca", name="oa", bufs=1)
                ob = ps.tile([128, 512], F32, tag="accb", name="ob", bufs=1)

                def scores(k, q0):
                    s2 = ps.tile([128, 1024], F32, tag="mm", name="s2")
                    for e in range(2):
                        rows = slice(64 * e, 64 * e + 64)
                        nc.tensor.matmul(
                            s2[:, e * 512 + q0 : (e + 1) * 512],
                            kt2[p][rows, k * 128 : (k + 1) * 128],
                            qt2[p][rows, qb * 512 + q0 : (qb + 1) * 512],
                            start=True,
                            stop=True,
                            tile_position=(64 * e, 0),
                        )
                    return s2

                def exp_full(s2, ptile, half):
                    nc.scalar.activation(
                        ptile[:, half, :],
                        s2[:],
                        mybir.ActivationFunctionType.Exp,
                        bias=bias_sb[:],
                        scale=float(EXP_SCALE),
                    )

                def exp_schraudolph(s2, ptile, half):
                    # 2^(z*log2e) via int bit trick on DVE: frees ScalarE
                    pi = pc.tile([128, 1024], mybir.dt.int32, tag="schr",
                                 name="pi", bufs=2)
                    A = float(EXP_SCALE) * 12102203.161561485  # 2^23/ln2
                    Bc = -2.0 * 12102203.161561485 + 127.0 * 8388608.0 - 361007.0
                    nc.vector.tensor_scalar(
                        pi[:], s2[:], A, Bc,
                        op0=mybir.AluOpType.mult, op1=mybir.AluOpType.add,
                    )
                    nc.vector.tensor_copy(ptile[:, half, :], pi[:].bitcast(F32))

                def exp_strided(s2, ptile, half, q0a):
                    src_ap = s2[:].rearrange("p (h q) -> p h q", h=2)[:, :, q0a:512]
                    dst = (
                        ptile[:, half, :]
                        .rearrange("p (h q) -> p h q", h=2)[:, :, q0a:512]
                    )
                    nc.scalar.activation(
                        dst,
                        src_ap,
                        mybir.ActivationFunctionType.Exp,
                        bias=bias_sb[:],
                        scale=float(EXP_SCALE),
                    )

                def mask_strip(ptile, half, q0a, zt):
                    width = 256 if zt else 128
                    mt = trizt_sb if zt else tri_sb
                    strip = (
                        ptile[:, half, :]
                        .rearrange("p (h q) -> p h q", h=2)[:, :, q0a : q0a + width]
                    )
                    nc.gpsimd.tensor_mul(
                        strip, strip, mt[:].rearrange("p (h q) -> p h q", h=2)
                    )

                def av_pair(k0, ptile, q0a, start, stop):
                    for e, o in ((0, oa), (1, ob)):
                        nc.tensor.matmul(
                            o[:, q0a:512],
                            v2e[p][:, k0 : k0 + 2, 128 * e : 128 * e + 128],
                            ptile[:, :, e * 512 + q0a : (e + 1) * 512],
                            start=start,
                            stop=stop,
                            perf_mode=DR,
                        )

                # units: DoubleRow pairs; non-crossing pairs full-width, the
                # two crossing pairs trimmed to q0a with [zero|tri] masking on
                # the odd member.  AV issue lags scores by 2 units so the PE
                # never stalls on the exp/mask chain.  Some non-crossing exps
                # run as Schraudolph fast-exp on DVE to unload ScalarE.
                units = [("nc", j) for j in range(2 * qb)]
                units += [("cr", r) for r in range(2)]
                tiles = {}
                pending = []
                nunit = len(units)

                def issue_av(idx):
                    kind, a = units[idx]
                    start = idx == 0
                    stop = idx == nunit - 1
                    if kind == "nc":
                        av_pair(2 * a, tiles[idx], 0, start, stop)
                    else:
                        av_pair(4 * qb + 2 * a, tiles[idx], 256 * a, start, stop)

                for idx, (kind, a) in enumerate(units):
                    ptile = pc.tile([128, 2, 1024], F8, tag="pt", name="pt", bufs=6)
                    tiles[idx] = ptile
                    if kind == "nc":
                        offload = qb >= 2 and a in SCHRAUD.get(qb, ())
                        for half in range(2):
                            s2 = scores(2 * a + half, 0)
                            if offload:
                                exp_schraudolph(s2, ptile, half)
                            else:
                                exp_full(s2, ptile, half)
                    else:
                        q0a = 256 * a
                        for half in range(2):
                            s2 = scores(4 * qb + 2 * a + half, q0a)
                            exp_strided(s2, ptile, half, q0a)
                            mask_strip(ptile, half, q0a, zt=(half == 1))
                    pending.append(idx)
                    if filler is not None:
                        next(filler, None)
                    if len(pending) > 3:
                        issue_av(pending.pop(0))
                for idx in pending:
                    issue_av(idx)
                if filler is not None:
                    for _ in filler:
                        pass

                stage = pc.tile([65, 2, 512], F16, tag="stage", name="stage")
                nc.vector.tensor_copy(stage[:, 0, :], oa[0:65, :])
                nc.vector.tensor_copy(stage[:, 1, :], ob[0:65, :])
                nc.sync.dma_start(
                    out=out.rearrange("h (nl q) -> h nl q", nl=HPC)[
                        :, 2 * p : 2 * p + 2, qb * 512 : (qb + 1) * 512
                    ],
                    in_=stage[:],
                )

            # interleave: the second projection chunk's steps are spread
            # between the early attention blocks so projection matmuls fill
            # the PE while ScalarE drains the attention exps (and vice versa)
            import itertools

            for p in range(NPAIR):
                for t in range(3):
                    proj_step(0, p, t)
            attention(0, 1, filler=proj_step_gen(1, 0, 0))
            attention(1, 1, filler=proj_step_gen(1, 0, 1))
            attention(0, 0, filler=proj_step_gen(1, 0, 2))
            attention(1, 0, filler=proj_step_gen(1, 1, 0))
            attention(
                0, 3,
                filler=itertools.chain(
                    proj_step_gen(1, 1, 1), proj_step_gen(1, 1, 2)
                ),
            )
            attention(1, 3)
            attention(0, 2)
            attention(1, 2)

    if split_waits:
        _split_excess_waits(nc)
    return nc


_NC_CACHE = None


def _get_nc():
    global _NC_CACHE
    if _NC_CACHE is None:
        _NC_CACHE = build_nc()
    return _NC_CACHE


def _host_inputs(x, W_qkv):
    """Per-core input maps."""
    xT8 = [np.ascontiguousarray(x[b].T).astype(NP8) for b in range(B)]
    # w[pair, t, dtile, 128, 128]: cols 0:64 head a, 64:128 head b
    Wr = np.ascontiguousarray((W_qkv * WSCALE).reshape(NH, DT, 128, 3, HD))
    ki = np.arange(128)[:, None]
    qj = np.arange(128)[None, :]
    tri1 = (ki <= qj).astype(np.float32)
    tri = np.concatenate([tri1, tri1], axis=1).astype(NP8)  # [128, 256]
    z1 = np.zeros((128, 128), dtype=np.float32)
    trizt = np.concatenate([z1, tri1, z1, tri1], axis=1).astype(NP8)  # [128, 512]
    ident = np.eye(128, dtype=np.float16)
    in_maps = []
    for c in range(NCORES):
        b = c // 4
        h0 = 4 * (c % 4)
        w = np.empty((NPAIR, 3, DT, 128, 128), dtype=np.float32)
        for p in range(NPAIR):
            ha, hb = h0 + 2 * p, h0 + 2 * p + 1
            for t in range(3):
                w[p, t, :, :, 0:64] = Wr[ha, :, :, t, :]
                w[p, t, :, :, 64:128] = Wr[hb, :, :, t, :]
        in_maps.append(
            {"xT": xT8[b], "w": w.astype(NP8), "tri": tri, "trizt": trizt, "ident": ident}
        )
    return in_maps


def _patch_early_rows(O, x, W_qkv):
    """Exact recompute of rows < PATCH_ROWS: with tiny softmax support the
    fp8 V/P quantization error doesn't average out there."""
    R = PATCH_ROWS
    if not R:
        return
    xr = x[:, :R].astype(np.float64)  # [B, R, D]
    W = W_qkv.astype(np.float64)
    qkv = np.einsum("brd,ndh->bnrh", xr, W)  # [B, N, R, 3H]
    q, k, v = np.split(qkv, 3, axis=-1)
    sc = np.einsum("bnrh,bnsh->bnrs", q, k) / np.sqrt(HD)
    tril = np.tril(np.ones((R, R)))
    P = np.exp(sc) * tril[None, None]
    o = P @ v / P.sum(-1)[..., None]  # [B, N, R, H]
    O[:, :, :R, :] = o.astype(np.float32)


def _host_epilogue(results, x, W_qkv, W_out):
    # V was computed from W_v*16, so the numerator rows carry a 16x factor.
    W_sum = W_out.sum(axis=(0, 1)).astype(np.float32)  # [D]
    O = np.empty((B, NH, S, HD), dtype=np.float32)
    for c in range(NCORES):
        o = results[c]["out"].astype(np.float32)  # [65, 4*2048]
        b = c // 4
        h0 = 4 * (c % 4)
        body = o[0:64].reshape(64, HPC, S)  # [h, nl, s]
        den = o[64].reshape(HPC, S)  # [nl, s]
        O[b, h0 : h0 + HPC] = body.transpose(1, 2, 0) / (WSCALE * den[:, :, None])
    _patch_early_rows(O, x, W_qkv)
    out2 = O.reshape(B, D, S)  # raw row-major reshape, as in the reference
    return np.ascontiguousarray(
        out2.transpose(0, 2, 1) * W_sum[None, None, :]
    ).astype(np.float32)


def _run(x, W_qkv, W_out, trace=False):
    nc = _get_nc()
    in_maps = _host_inputs(x, W_qkv)
    res = run_bass_kernel_spmd(
        nc,
        in_maps,
        list(range(NCORES)),
        trace=trace,
        trace_cores=list(range(NCORES)) if trace else None,
    )
    return _host_epilogue(res.results, x, W_qkv, W_out), res


def kernel(x, W_qkv, W_out):
    x = np.asarray(x, dtype=np.float32)
    W_qkv = np.asarray(W_qkv, dtype=np.float32)
    W_out = np.asarray(W_out, dtype=np.float32)
    out, _ = _run(x, W_qkv, W_out, trace=False)
    return out


def kernel_traced(x, W_qkv, W_out):
    out, res = _run(
        np.asarray(x, np.float32),
        np.asarray(W_qkv, np.float32),
        np.asarray(W_out, np.float32),
        trace=True,
    )
    return out, res


# revision 27
# speedup vs baseline: 1.2212x; 1.2212x over previous
"""Causal multi-head attention kernel for TRN2 (8 NeuronCores, SPMD).

Problem: x[2,2048,1024], per-head W_qkv[16,1024,192], W_out[16,64,1024].
  qkv = einsum('bsd,ndh->bnsh', x, W_qkv); causal softmax attention per head;
  out.reshape(B,-1,S); einsum('bds,nhd->bsd', out, W_out).

The final einsum does NOT contract d (it appears in both operands and the
output), so it reduces to a per-column scale by W_sum[d] = sum_{n,h} W_out —
pure data movement, done on the host.  The device computes the attention.

Sharding: 2 batches x 16 heads = 32 jobs; core c handles batch c//4 and the
4 heads [4*(c%4), 4*(c%4)+4), as 2 head-pairs packed into 128 partitions.

Device per core:
  - QKV projection in fp8e4 DoubleRow (x fp8; W fp8 pre-scaled x16 past the
    e4m3 subnormal floor): two 128-row d-subtiles per matmul pass.
  - Q^T/K^T evacuated fp16 (scores fp16: x16 scale cancels via exp scale).
  - V^T via fp16 PE transpose into [k, Va|1|0.. , Vb|1|0..] fp8 tiles
    (power-of-2 128-wide per head for fp8 LDWEIGHTS ISA rules); the ones
    column makes the AV matmul also produce the softmax denominator.
  - scores: per k-tile, 2 fp16 matmuls (heads packed via tile_position)
    into one [128,1024] fp32 PSUM tile; ONE exp per k-tile on ScalarE with
    scale=1/(8*256), bias=-2 (keeps exp inside the e4m3 +-240 range;
    cancels in normalization), writing fp8 P directly.
  - causal mask: only the 128-col diagonal strip of a crossing tile needs
    element masking (one shared triangle for every rel) — strided fp8
    tensor_mul on the otherwise idle GpSimd engine.
  - AV in fp8: non-crossing k-tiles accumulate two-at-a-time with DoubleRow
    ([128,2,128] stationary); crossing tiles as plain fp8 matmuls with
    causal column trim.  AV issue lags scores by 2 units so the PE never
    stalls on the ScalarE exp / GpSimd mask chain.
  - O'^T: rows 0..63 attention output (x16 from W scaling), row 64
    denominator; staged fp16 and DMA'd out per (pair, q-block).
  - PE warmup matmuls cover the initial DMA wait (p-state ramp).
Host epilogue: normalize, un-scale, reshape, scale by W_sum; rows < 256
(numerically degenerate for fp8: tiny softmax support exposes single-element
V/P quantization directly) are recomputed exactly — 1.5% of attention flops.
"""

import numpy as np
import ml_dtypes

import concourse.bass as bass
import concourse.mybir as mybir
from concourse.tile import TileContext
from concourse.bass_utils import run_bass_kernel_spmd

F32 = mybir.dt.float32
F16 = mybir.dt.float16
F8 = mybir.dt.float8e4
NP8 = ml_dtypes.float8_e4m3
DR = mybir.MatmulPerfMode.DoubleRow

B, S, D, NH, HD = 2, 2048, 1024, 16, 64  # batch, seq, model, heads, head_dim
NCORES = 8
HPC = 4  # heads per core
NPAIR = 2  # head pairs per core
DT = D // 128  # 8 D-tiles
NQB = S // 512  # 4 q blocks
NKT = S // 128  # 16 k tiles
WSCALE = 16.0  # host pre-scale on W_qkv to lift fp8 subnormals
EXP_SCALE = (1.0 / np.sqrt(HD)) / (WSCALE * WSCALE)
EXP_BIAS = -2.0
PATCH_ROWS = 256
# non-crossing pair indices whose exp runs on DVE (Schraudolph) per q-block
SCHRAUD = {2: (1,), 3: (1, 3, 5)}


def _split_excess_waits(nc, limit=1):
    """This walrus build rejects >1 sync-wait per instruction; hoist extra
    waits onto preceding same-engine no-ops (identical blocking semantics)."""
    cnt = 0
    for fn in nc.m.functions:
        for blk in fn.blocks:
            out = []
            for inst in blk.instructions:
                si = inst.sync_info
                if si is not None and si.on_wait and len(si.on_wait) > limit:
                    waits = list(si.on_wait)
                    excess, keep = waits[:-limit], waits[-limit:]
                    for i in range(0, len(excess), limit):
                        nop = mybir.InstNoOp(
                            name=f"wsplit_{cnt}", ins=[], outs=[], engine=inst.engine
                        )
                        cnt += 1
                        nop.sync_info = mybir.SyncInfo(
                            on_wait=excess[i : i + limit], on_update=[]
                        )
                        out.append(nop)
                    inst.sync_info = mybir.SyncInfo(
                        on_wait=keep, on_update=list(si.on_update or [])
                    )
                out.append(inst)
            blk.instructions = out
    return cnt


def build_nc(split_waits=True):
    nc = bass.Bass()
    xT = nc.declare_dram_parameter("xT", [D, S], F8, isOutput=False)
    w = nc.declare_dram_parameter("w", [NPAIR, 3, DT, 128, 128], F8, isOutput=False)
    tri = nc.declare_dram_parameter("tri", [128, 256], F8, isOutput=False)
    trizt = nc.declare_dram_parameter("trizt", [128, 512], F8, isOutput=False)
    ident = nc.declare_dram_parameter("ident", [128, 128], F16, isOutput=False)
    out = nc.declare_dram_parameter("out", [65, HPC * S], F16, isOutput=True)

    with TileContext(nc) as tc:
        with (
            tc.tile_pool(name="persist", bufs=1) as pp,
            tc.tile_pool(name="psum", bufs=3, space="PSUM") as ps,
            tc.tile_pool(name="work", bufs=2) as pc,
        ):
            # ---- persistent SBUF tensors (Q^T and K^T head-packed [2H, S])
            qt2 = [
                pp.tile([128, S], F16, tag=f"qt{p}", name=f"qtt{p}")
                for p in range(NPAIR)
            ]
            kt2 = [
                pp.tile([128, S], F16, tag=f"kt{p}", name=f"ktt{p}")
                for p in range(NPAIR)
            ]
            v2e = [
                pp.tile([128, NKT, 256], F8, tag=f"v2e{p}", name=f"v2e{p}")
                for p in range(NPAIR)
            ]
            tri_sb = pp.tile([128, 256], F8, tag="tri", name="tri_sb")
            trizt_sb = pp.tile([128, 512], F8, tag="trizt", name="trizt_sb")
            ident_sb = pp.tile([128, 128], F16, tag="ident", name="ident_sb")
            xt_sb = pp.tile([128, DT, S], F8, tag="xt", name="xt_sb")
            w_sb = pp.tile([128, NPAIR * 3 * DT, 128], F8, tag="w", name="w_sb")
            vt = [
                pp.tile([128, S], F16, tag=f"vt{p}", name=f"vt{p}")
                for p in range(NPAIR)
            ]
            vtT = [
                pp.tile([128, NKT, 128], F16, tag=f"vtT{p}", name=f"vtT{p}")
                for p in range(NPAIR)
            ]
            warm_sb = pp.tile([128, 640], F16, tag="warm", name="warm_sb")
            bias_sb = pp.tile([128, 1], F32, tag="bias", name="bias_sb")

            # engine-local constants / warmup source (no DMA dependency)
            nc.vector.memset(warm_sb[:], 0.0)
            nc.vector.memset(bias_sb[:], EXP_BIAS)
            for p in range(NPAIR):
                nc.vector.memset(v2e[p][:, :, 64], 1.0)
                nc.vector.memset(v2e[p][:, :, 192], 1.0)
                nc.vector.memset(v2e[p][:, :, 65:128], 0.0)
                nc.vector.memset(v2e[p][:, :, 193:256], 0.0)

            # PE warmup: keep the PE busy during the input DMA so the
            # p-state ramp completes before the first real matmul.
            wt = ps.tile([128, 512], F32, tag="mm", name="warmup")
            for _ in range(10):
                nc.tensor.matmul(
                    wt[:], warm_sb[:, 0:128], warm_sb[:, 128:640],
                    start=True, stop=True,
                )

            # DMA order = consumption order: pair-0 Q weights + first xT half
            # gate the first projection matmuls.
            w_v = w.rearrange("a t d k m -> k (a t d) m")
            xt_v = xT.rearrange("(dt p) s -> p dt s", p=128)
            nc.sync.dma_start(out=xt_sb[:, :, 0:512], in_=xt_v[:, :, 0:512])
            nc.sync.dma_start(out=w_sb[:, 0:DT, :], in_=w_v[:, 0:DT, :])
            nc.sync.dma_start(
                out=w_sb[:, DT : 3 * DT, :], in_=w_v[:, DT : 3 * DT, :]
            )
            nc.sync.dma_start(
                out=w_sb[:, 3 * DT : 6 * DT, :], in_=w_v[:, 3 * DT : 6 * DT, :]
            )
            nc.scalar.dma_start(out=xt_sb[:, :, 512:1024], in_=xt_v[:, :, 512:1024])
            nc.scalar.dma_start(out=xt_sb[:, :, 1024:1536], in_=xt_v[:, :, 1024:1536])
            nc.scalar.dma_start(out=xt_sb[:, :, 1536:2048], in_=xt_v[:, :, 1536:2048])
            nc.gpsimd.dma_start(out=ident_sb[:], in_=ident[:])
            nc.gpsimd.dma_start(out=tri_sb[:], in_=tri[:])
            nc.gpsimd.dma_start(out=trizt_sb[:], in_=trizt[:])

            def proj_step_gen(qcp, p, t):
                """Project one (pair, qkv-type) for q-columns
                [qcp*1024, (qcp+1)*1024) with fp8 DoubleRow (2 d-subtiles per
                pass); transpose V k-tiles of the chunk.  Yields between
                pieces so callers can interleave it into attention blocks."""
                acc2 = ps.tile([128, 1024], F32, tag="mm", name="acc2")
                for dp in range(DT // 2):
                    base = (p * 3 + t) * DT + 2 * dp
                    wsl = w_sb[:, base : base + 2, :]
                    for h in range(2):
                        qc = 2 * qcp + h
                        nc.tensor.matmul(
                            acc2[:, h * 512 : (h + 1) * 512],
                            wsl,
                            xt_sb[:, 2 * dp : 2 * dp + 2, qc * 512 : (qc + 1) * 512],
                            start=(dp == 0),
                            stop=(dp == DT // 2 - 1),
                            perf_mode=DR,
                        )
                csl = slice(qcp * 1024, (qcp + 1) * 1024)
                if t == 0:
                    nc.vector.tensor_copy(qt2[p][:, csl], acc2[:])
                    yield
                elif t == 1:
                    nc.vector.tensor_copy(kt2[p][:, csl], acc2[:])
                    yield
                else:  # V -> fp16; transpose the chunk off-PE via DMA xbar
                    nc.vector.tensor_copy(vt[p][:, csl], acc2[:])
                    nc.sync.dma_start_transpose(
                        vtT[p][:, 8 * qcp : 8 * qcp + 8, :], vt[p][:, csl]
                    )
                    yield
                    for k in range(8 * qcp, 8 * qcp + 8):
                        if k % 4 == 2:
                            yield
                        # one strided fp16->fp8 copy into the padded layout
                        nc.gpsimd.tensor_copy(
                            v2e[p][:, k, :].rearrange("p (t c) -> p t c", t=2)[
                                :, :, 0:64
                            ],
                            vtT[p][:, k, :].rearrange("p (t c) -> p t c", t=2),
                        )

            def proj_step(qcp, p, t):
                for _ in proj_step_gen(qcp, p, t):
                    pass

            def attention(p, qb, filler=None):
                nk = 4 * (qb + 1)  # causal: k tiles 0..nk-1
                oa = ps.tile([128, 512], F32, tag="acca", name="oa", bufs=1)
                ob = ps.tile([128, 512], F32, tag="accb", name="ob", bufs=1)

                def scores(k, q0):
                    s2 = ps.tile([128, 1024], F32, tag="mm", name="s2")
                    for e in range(2):
                        rows = slice(64 * e, 64 * e + 64)
                        nc.tensor.matmul(
                            s2[:, e * 512 + q0 : (e + 1) * 512],
                            kt2[p][rows, k * 128 : (k + 1) * 128],
                            qt2[p][rows, qb * 512 + q0 : (qb + 1) * 512],
                            start=True,
                            stop=True,
                            tile_position=(64 * e, 0),
                        )
                    return s2

                def exp_full(s2, ptile, half):
                    nc.scalar.activation(
                        ptile[:, half, :],
                        s2[:],
                        mybir.ActivationFunctionType.Exp,
                        bias=bias_sb[:],
                        scale=float(EXP_SCALE),
                    )

                def exp_schraudolph(s2, ptile, half):
                    # 2^(z*log2e) via int bit trick on DVE: frees ScalarE
                    pi = pc.tile([128, 1024], mybir.dt.int32, tag="schr",
                                 name="pi", bufs=2)
                    A = float(EXP_SCALE) * 12102203.161561485  # 2^23/ln2
                    Bc = -2.0 * 12102203.161561485 + 127.0 * 8388608.0 - 361007.0
                    nc.vector.tensor_scalar(
                        pi[:], s2[:], A, Bc,
                        op0=mybir.AluOpType.mult, op1=mybir.AluOpType.add,
                    )
                    nc.vector.tensor_copy(ptile[:, half, :], pi[:].bitcast(F32))

                def exp_strided(s2, ptile, half, q0a):
                    src_ap = s2[:].rearrange("p (h q) -> p h q", h=2)[:, :, q0a:512]
                    dst = (
                        ptile[:, half, :]
                        .rearrange("p (h q) -> p h q", h=2)[:, :, q0a:512]
                    )
                    nc.scalar.activation(
                        dst,
                        src_ap,
                        mybir.ActivationFunctionType.Exp,
                        bias=bias_sb[:],
                        scale=float(EXP_SCALE),
                    )

                def mask_strip(ptile, half, q0a, zt):
                    width = 256 if zt else 128
                    mt = trizt_sb if zt else tri_sb
                    strip = (
                        ptile[:, half, :]
                        .rearrange("p (h q) -> p h q", h=2)[:, :, q0a : q0a + width]
                    )
                    nc.gpsimd.tensor_mul(
                        strip, strip, mt[:].rearrange("p (h q) -> p h q", h=2)
                    )

                def av_pair(k0, ptile, q0a, start, stop):
                    for e, o in ((0, oa), (1, ob)):
                        nc.tensor.matmul(
                            o[:, q0a:512],
                            v2e[p][:, k0 : k0 + 2, 128 * e : 128 * e + 128],
                            ptile[:, :, e * 512 + q0a : (e + 1) * 512],
                            start=start,
                            stop=stop,
                            perf_mode=DR,
                        )

                # units: DoubleRow pairs; non-crossing pairs full-width, the
                # two crossing pairs trimmed to q0a with [zero|tri] masking on
                # the odd member.  AV issue lags scores by 2 units so the PE
                # never stalls on the exp/mask chain.  Some non-crossing exps
                # run as Schraudolph fast-exp on DVE to unload ScalarE.
                units = [("nc", j) for j in range(2 * qb)]
                units += [("cr", r) for r in range(2)]
                tiles = {}
                pending = []
                nunit = len(units)

                def issue_av(idx):
                    kind, a = units[idx]
                    start = idx == 0
                    stop = idx == nunit - 1
                    if kind == "nc":
                        av_pair(2 * a, tiles[idx], 0, start, stop)
                    else:
                        av_pair(4 * qb + 2 * a, tiles[idx], 256 * a, start, stop)

                for idx, (kind, a) in enumerate(units):
                    ptile = pc.tile([128, 2, 1024], F8, tag="pt", name="pt", bufs=6)
                    tiles[idx] = ptile
                    if kind == "nc":
                        offload = qb >= 2 and a in SCHRAUD.get(qb, ())
                        for half in range(2):
                            s2 = scores(2 * a + half, 0)
                            if offload:
                                exp_schraudolph(s2, ptile, half)
                            else:
                                exp_full(s2, ptile, half)
                    else:
                        q0a = 256 * a
                        for half in range(2):
                            s2 = scores(4 * qb + 2 * a + half, q0a)
                            exp_strided(s2, ptile, half, q0a)
                            mask_strip(ptile, half, q0a, zt=(half == 1))
                    pending.append(idx)
                    if filler is not None:
                        next(filler, None)
                    if len(pending) > 3:
                        issue_av(pending.pop(0))
                for idx in pending:
                    issue_av(idx)
                if filler is not None:
                    for _ in filler:
                        pass

                stage = pc.tile([65, 2, 512], F16, tag="stage", name="stage")
                nc.vector.tensor_copy(stage[:, 0, :], oa[0:65, :])
                nc.vector.tensor_copy(stage[:, 1, :], ob[0:65, :])
                nc.sync.dma_start(
                    out=out.rearrange("h (nl q) -> h nl q", nl=HPC)[
                        :, 2 * p : 2 * p + 2, qb * 512 : (qb + 1) * 512
                    ],
                    in_=stage[:],
                )

            # interleave: the second projection chunk's steps are spread
            # between the early attention blocks so projection matmuls fill
            # the PE while ScalarE drains the attention exps (and vice versa)
            import itertools

            for p in range(NPAIR):
                for t in range(3):
                    proj_step(0, p, t)
            attention(0, 1, filler=proj_step_gen(1, 0, 0))
            attention(1, 1, filler=proj_step_gen(1, 0, 1))
            attention(0, 0, filler=proj_step_gen(1, 0, 2))
            attention(1, 0, filler=proj_step_gen(1, 1, 0))
            attention(
                0, 3,
                filler=itertools.chain(
                    proj_step_gen(1, 1, 1), proj_step_gen(1, 1, 2)
                ),
            )
            attention(1, 3)
            attention(0, 2)
            attention(1, 2)

    if split_waits:
        _split_excess_waits(nc)
    return nc


_NC_CACHE = None


def _get_nc():
    global _NC_CACHE
    if _NC_CACHE is None:
        _NC_CACHE = build_nc()
    return _NC_CACHE


def _host_inputs(x, W_qkv):
    """Per-core input maps."""
    xT8 = [np.ascontiguousarray(x[b].T).astype(NP8) for b in range(B)]
    # w[pair, t, dtile, 128, 128]: cols 0:64 head a, 64:128 head b
    Wr = np.ascontiguousarray((W_qkv * WSCALE).reshape(NH, DT, 128, 3, HD))
    ki = np.arange(128)[:, None]
    qj = np.arange(128)[None, :]
    tri1 = (ki <= qj).astype(np.float32)
    tri = np.concatenate([tri1, tri1], axis=1).astype(NP8)  # [128, 256]
    z1 = np.zeros((128, 128), dtype=np.float32)
    trizt = np.concatenate([z1, tri1, z1, tri1], axis=1).astype(NP8)  # [128, 512]
    ident = np.eye(128, dtype=np.float16)
    in_maps = []
    for c in range(NCORES):
        b = c // 4
        h0 = 4 * (c % 4)
        w = np.empty((NPAIR, 3, DT, 128, 128), dtype=np.float32)
        for p in range(NPAIR):
            ha, hb = h0 + 2 * p, h0 + 2 * p + 1
            for t in range(3):
                w[p, t, :, :, 0:64] = Wr[ha, :, :, t, :]
                w[p, t, :, :, 64:128] = Wr[hb, :, :, t, :]
        in_maps.append(
            {"xT": xT8[b], "w": w.astype(NP8), "tri": tri, "trizt": trizt, "ident": ident}
        )
    return in_maps


def _patch_early_rows(O, x, W_qkv):
    """Exact recompute of rows < PATCH_ROWS: with tiny softmax support the
    fp8 V/P quantization error doesn't average out there."""
    R = PATCH_ROWS
    if not R:
        return
    xr = x[:, :R].astype(np.float64)  # [B, R, D]
    W = W_qkv.astype(np.float64)
    qkv = np.einsum("brd,ndh->bnrh", xr, W)  # [B, N, R, 3H]
    q, k, v = np.split(qkv, 3, axis=-1)
    sc = np.einsum("bnrh,bnsh->bnrs", q, k) / np.sqrt(HD)
    tril = np.tril(np.ones((R, R)))
    P = np.exp(sc) * tril[None, None]
    o = P @ v / P.sum(-1)[..., None]  # [B, N, R, H]
    O[:, :, :R, :] = o.astype(np.float32)


def _host_epilogue(results, x, W_qkv, W_out):
    # V was computed from W_v*16, so the numerator rows carry a 16x factor.
    W_sum = W_out.sum(axis=(0, 1)).astype(np.float32)  # [D]
    O = np.empty((B, NH, S, HD), dtype=np.float32)
    for c in range(NCORES):
        o = results[c]["out"].astype(np.float32)  # [65, 4*2048]
        b = c // 4
        h0 = 4 * (c % 4)
        body = o[0:64].reshape(64, HPC, S)  # [h, nl, s]
        den = o[64].reshape(HPC, S)  # [nl, s]
        O[b, h0 : h0 + HPC] = body.transpose(1, 2, 0) / (WSCALE * den[:, :, None])
    _patch_early_rows(O, x, W_qkv)
    out2 = O.reshape(B, D, S)  # raw row-major reshape, as in the reference
    return np.ascontiguousarray(
        out2.transpose(0, 2, 1) * W_sum[None, None, :]
    ).astype(np.float32)


def _run(x, W_qkv, W_out, trace=False):
    nc = _get_nc()
    in_maps = _host_inputs(x, W_qkv)
    res = run_bass_kernel_spmd(
        nc,
        in_maps,
        list(range(NCORES)),
        trace=trace,
        trace_cores=list(range(NCORES)) if trace else None,
    )
    return _host_epilogue(res.results, x, W_qkv, W_out), res


def kernel(x, W_qkv, W_out):
    x = np.asarray(x, dtype=np.float32)
    W_qkv = np.asarray(W_qkv, dtype=np.float32)
    W_out = np.asarray(W_out, dtype=np.float32)
    out, _ = _run(x, W_qkv, W_out, trace=False)
    return out


def kernel_traced(x, W_qkv, W_out):
    out, res = _run(
        np.asarray(x, np.float32),
        np.asarray(W_qkv, np.float32),
        np.asarray(W_out, np.float32),
        trace=True,
    )
    return out, res


# revision 28
# speedup vs baseline: 1.2933x; 1.0590x over previous
"""Causal multi-head attention kernel for TRN2 (8 NeuronCores, SPMD).

Problem: x[2,2048,1024], per-head W_qkv[16,1024,192], W_out[16,64,1024].
  qkv = einsum('bsd,ndh->bnsh', x, W_qkv); causal softmax attention per head;
  out.reshape(B,-1,S); einsum('bds,nhd->bsd', out, W_out).

The final einsum does NOT contract d (it appears in both operands and the
output), so it reduces to a per-column scale by W_sum[d] = sum_{n,h} W_out —
pure data movement, done on the host.  The device computes the attention.

Sharding: 2 batches x 16 heads = 32 jobs; core c handles batch c//4 and the
4 heads [4*(c%4), 4*(c%4)+4), as 2 head-pairs packed into 128 partitions.

Device per core:
  - QKV projection in fp8e4 DoubleRow (x fp8; W fp8 pre-scaled x16 past the
    e4m3 subnormal floor): two 128-row d-subtiles per matmul pass.
  - Q^T/K^T evacuated fp16 (scores fp16: x16 scale cancels via exp scale).
  - V^T via fp16 PE transpose into [k, Va|1|0.. , Vb|1|0..] fp8 tiles
    (power-of-2 128-wide per head for fp8 LDWEIGHTS ISA rules); the ones
    column makes the AV matmul also produce the softmax denominator.
  - scores: per k-tile, 2 fp16 matmuls (heads packed via tile_position)
    into one [128,1024] fp32 PSUM tile; ONE exp per k-tile on ScalarE with
    scale=1/(8*256), bias=-2 (keeps exp inside the e4m3 +-240 range;
    cancels in normalization), writing fp8 P directly.
  - causal mask: only the 128-col diagonal strip of a crossing tile needs
    element masking (one shared triangle for every rel) — strided fp8
    tensor_mul on the otherwise idle GpSimd engine.
  - AV in fp8: non-crossing k-tiles accumulate two-at-a-time with DoubleRow
    ([128,2,128] stationary); crossing tiles as plain fp8 matmuls with
    causal column trim.  AV issue lags scores by 2 units so the PE never
    stalls on the ScalarE exp / GpSimd mask chain.
  - O'^T: rows 0..63 attention output (x16 from W scaling), row 64
    denominator; staged fp16 and DMA'd out per (pair, q-block).
  - PE warmup matmuls cover the initial DMA wait (p-state ramp).
Host epilogue: normalize, un-scale, reshape, scale by W_sum; rows < 256
(numerically degenerate for fp8: tiny softmax support exposes single-element
V/P quantization directly) are recomputed exactly — 1.5% of attention flops.
"""

import numpy as np
import ml_dtypes

import concourse.bass as bass
import concourse.mybir as mybir
from concourse.tile import TileContext
from concourse.bass_utils import run_bass_kernel_spmd

F32 = mybir.dt.float32
F16 = mybir.dt.float16
F8 = mybir.dt.float8e4
NP8 = ml_dtypes.float8_e4m3
DR = mybir.MatmulPerfMode.DoubleRow

B, S, D, NH, HD = 2, 2048, 1024, 16, 64  # batch, seq, model, heads, head_dim
NCORES = 8
HPC = 4  # heads per core
NPAIR = 2  # head pairs per core
DT = D // 128  # 8 D-tiles
NQB = S // 512  # 4 q blocks
NKT = S // 128  # 16 k tiles
WSCALE = 16.0  # host pre-scale on W_qkv to lift fp8 subnormals
EXP_SCALE = (1.0 / np.sqrt(HD)) / (WSCALE * WSCALE)
EXP_BIAS = -2.0
PATCH_ROWS = 256
# non-crossing pair indices whose exp runs on DVE (Schraudolph) per q-block
SCHRAUD = {2: (1,), 3: (1, 3, 5)}


def _split_excess_waits(nc, limit=1):
    """This walrus build rejects >1 sync-wait per instruction; hoist extra
    waits onto preceding same-engine no-ops (identical blocking semantics)."""
    cnt = 0
    for fn in nc.m.functions:
        for blk in fn.blocks:
            out = []
            for inst in blk.instructions:
                si = inst.sync_info
                if si is not None and si.on_wait and len(si.on_wait) > limit:
                    waits = list(si.on_wait)
                    excess, keep = waits[:-limit], waits[-limit:]
                    for i in range(0, len(excess), limit):
                        nop = mybir.InstNoOp(
                            name=f"wsplit_{cnt}", ins=[], outs=[], engine=inst.engine
                        )
                        cnt += 1
                        nop.sync_info = mybir.SyncInfo(
                            on_wait=excess[i : i + limit], on_update=[]
                        )
                        out.append(nop)
                    inst.sync_info = mybir.SyncInfo(
                        on_wait=keep, on_update=list(si.on_update or [])
                    )
                out.append(inst)
            blk.instructions = out
    return cnt


def build_nc(split_waits=True):
    nc = bass.Bass()
    xT = nc.declare_dram_parameter("xT", [D, S], F8, isOutput=False)
    w = nc.declare_dram_parameter("w", [NPAIR, 3, DT, 128, 128], F8, isOutput=False)
    tri = nc.declare_dram_parameter("tri", [128, 256], F8, isOutput=False)
    trizt = nc.declare_dram_parameter("trizt", [128, 512], F8, isOutput=False)
    ident = nc.declare_dram_parameter("ident", [128, 128], F16, isOutput=False)
    out = nc.declare_dram_parameter("out", [65, HPC * S], F16, isOutput=True)

    with TileContext(nc) as tc:
        with (
            tc.tile_pool(name="persist", bufs=1) as pp,
            tc.tile_pool(name="psum", bufs=3, space="PSUM") as ps,
            tc.tile_pool(name="work", bufs=2) as pc,
        ):
            # ---- persistent SBUF tensors (Q^T and K^T head-packed [2H, S])
            qt2 = [
                pp.tile([128, S], F16, tag=f"qt{p}", name=f"qtt{p}")
                for p in range(NPAIR)
            ]
            kt2 = [
                pp.tile([128, S], F16, tag=f"kt{p}", name=f"ktt{p}")
                for p in range(NPAIR)
            ]
            v2e = [
                pp.tile([128, NKT, 256], F8, tag=f"v2e{p}", name=f"v2e{p}")
                for p in range(NPAIR)
            ]
            tri_sb = pp.tile([128, 256], F8, tag="tri", name="tri_sb")
            trizt_sb = pp.tile([128, 512], F8, tag="trizt", name="trizt_sb")
            ident_sb = pp.tile([128, 128], F16, tag="ident", name="ident_sb")
            xt_sb = pp.tile([128, DT, S], F8, tag="xt", name="xt_sb")
            w_sb = pp.tile([128, NPAIR * 3 * DT, 128], F8, tag="w", name="w_sb")
            vt = [
                pp.tile([128, S], F16, tag=f"vt{p}", name=f"vt{p}")
                for p in range(NPAIR)
            ]
            vtT = [
                pp.tile([128, NKT, 128], F16, tag=f"vtT{p}", name=f"vtT{p}")
                for p in range(NPAIR)
            ]
            warm_sb = pp.tile([128, 640], F16, tag="warm", name="warm_sb")
            bias_sb = pp.tile([128, 1], F32, tag="bias", name="bias_sb")

            # engine-local constants / warmup source (no DMA dependency)
            nc.vector.memset(warm_sb[:], 0.0)
            nc.vector.memset(bias_sb[:], EXP_BIAS)
            for p in range(NPAIR):
                nc.vector.memset(v2e[p][:, :, 64], 1.0)
                nc.vector.memset(v2e[p][:, :, 192], 1.0)
                nc.vector.memset(v2e[p][:, :, 65:128], 0.0)
                nc.vector.memset(v2e[p][:, :, 193:256], 0.0)

            # PE warmup: keep the PE busy during the input DMA so the
            # p-state ramp completes before the first real matmul.
            wt = ps.tile([128, 512], F32, tag="mm", name="warmup")
            for _ in range(10):
                nc.tensor.matmul(
                    wt[:], warm_sb[:, 0:128], warm_sb[:, 128:640],
                    start=True, stop=True,
                )

            # DMA order = consumption order: pair-0 Q weights + first xT half
            # gate the first projection matmuls.
            w_v = w.rearrange("a t d k m -> k (a t d) m")
            xt_v = xT.rearrange("(dt p) s -> p dt s", p=128)
            nc.sync.dma_start(out=xt_sb[:, :, 0:512], in_=xt_v[:, :, 0:512])
            nc.sync.dma_start(out=w_sb[:, 0:DT, :], in_=w_v[:, 0:DT, :])
            nc.sync.dma_start(
                out=w_sb[:, DT : 3 * DT, :], in_=w_v[:, DT : 3 * DT, :]
            )
            nc.sync.dma_start(
                out=w_sb[:, 3 * DT : 6 * DT, :], in_=w_v[:, 3 * DT : 6 * DT, :]
            )
            nc.scalar.dma_start(out=xt_sb[:, :, 512:1024], in_=xt_v[:, :, 512:1024])
            nc.scalar.dma_start(out=xt_sb[:, :, 1024:1536], in_=xt_v[:, :, 1024:1536])
            nc.scalar.dma_start(out=xt_sb[:, :, 1536:2048], in_=xt_v[:, :, 1536:2048])
            nc.gpsimd.dma_start(out=ident_sb[:], in_=ident[:])
            nc.gpsimd.dma_start(out=tri_sb[:], in_=tri[:])
            nc.gpsimd.dma_start(out=trizt_sb[:], in_=trizt[:])

            def proj_step_gen(qcp, p, t):
                """Project one (pair, qkv-type) for q-columns
                [qcp*1024, (qcp+1)*1024) with fp8 DoubleRow (2 d-subtiles per
                pass); transpose V k-tiles of the chunk.  Yields between
                pieces so callers can interleave it into attention blocks."""
                acc2 = ps.tile([128, 1024], F32, tag="mm", name="acc2")
                for dp in range(DT // 2):
                    base = (p * 3 + t) * DT + 2 * dp
                    wsl = w_sb[:, base : base + 2, :]
                    for h in range(2):
                        qc = 2 * qcp + h
                        nc.tensor.matmul(
                            acc2[:, h * 512 : (h + 1) * 512],
                            wsl,
                            xt_sb[:, 2 * dp : 2 * dp + 2, qc * 512 : (qc + 1) * 512],
                            start=(dp == 0),
                            stop=(dp == DT // 2 - 1),
                            perf_mode=DR,
                        )
                csl = slice(qcp * 1024, (qcp + 1) * 1024)
                if t == 0:
                    nc.vector.tensor_copy(qt2[p][:, csl], acc2[:])
                    yield
                elif t == 1:
                    nc.vector.tensor_copy(kt2[p][:, csl], acc2[:])
                    yield
                else:  # V -> fp16; transpose the chunk off-PE via DMA xbar
                    nc.vector.tensor_copy(vt[p][:, csl], acc2[:])
                    nc.sync.dma_start_transpose(
                        vtT[p][:, 8 * qcp : 8 * qcp + 8, :], vt[p][:, csl]
                    )
                    yield
                    for k in range(8 * qcp, 8 * qcp + 8):
                        if k % 4 == 2:
                            yield
                        # one strided fp16->fp8 copy into the padded layout
                        nc.gpsimd.tensor_copy(
                            v2e[p][:, k, :].rearrange("p (t c) -> p t c", t=2)[
                                :, :, 0:64
                            ],
                            vtT[p][:, k, :].rearrange("p (t c) -> p t c", t=2),
                        )

            def proj_step(qcp, p, t):
                for _ in proj_step_gen(qcp, p, t):
                    pass

            def attention(p, qb, filler=None):
                nk = 4 * (qb + 1)  # causal: k tiles 0..nk-1
                oa = ps.tile([128, 512], F32, tag="acca", name="oa", bufs=1)
                ob = ps.tile([128, 512], F32, tag="accb", name="ob", bufs=1)

                def scores(k, q0):
                    s2 = ps.tile([128, 1024], F32, tag="mm", name="s2")
                    for e in range(2):
                        rows = slice(64 * e, 64 * e + 64)
                        nc.tensor.matmul(
                            s2[:, e * 512 + q0 : (e + 1) * 512],
                            kt2[p][rows, k * 128 : (k + 1) * 128],
                            qt2[p][rows, qb * 512 + q0 : (qb + 1) * 512],
                            start=True,
                            stop=True,
                            tile_position=(64 * e, 0),
                        )
                    return s2

                def exp_full(s2, ptile, half):
                    nc.scalar.activation(
                        ptile[:, half, :],
                        s2[:],
                        mybir.ActivationFunctionType.Exp,
                        bias=bias_sb[:],
                        scale=float(EXP_SCALE),
                    )

                def exp_schraudolph(s2, ptile, half):
                    # 2^(z*log2e) via int bit trick on DVE: frees ScalarE
                    pi = pc.tile([128, 1024], mybir.dt.int32, tag="schr",
                                 name="pi", bufs=2)
                    A = float(EXP_SCALE) * 12102203.161561485  # 2^23/ln2
                    Bc = -2.0 * 12102203.161561485 + 127.0 * 8388608.0 - 361007.0
                    nc.vector.tensor_scalar(
                        pi[:], s2[:], A, Bc,
                        op0=mybir.AluOpType.mult, op1=mybir.AluOpType.add,
                    )
                    nc.vector.tensor_copy(ptile[:, half, :], pi[:].bitcast(F32))

                def exp_strided(s2, ptile, half, q0a):
                    src_ap = s2[:].rearrange("p (h q) -> p h q", h=2)[:, :, q0a:512]
                    dst = (
                        ptile[:, half, :]
                        .rearrange("p (h q) -> p h q", h=2)[:, :, q0a:512]
                    )
                    nc.scalar.activation(
                        dst,
                        src_ap,
                        mybir.ActivationFunctionType.Exp,
                        bias=bias_sb[:],
                        scale=float(EXP_SCALE),
                    )

                def mask_strip(ptile, half, q0a, zt):
                    width = 256 if zt else 128
                    mt = trizt_sb if zt else tri_sb
                    strip = (
                        ptile[:, half, :]
                        .rearrange("p (h q) -> p h q", h=2)[:, :, q0a : q0a + width]
                    )
                    nc.gpsimd.tensor_mul(
                        strip, strip, mt[:].rearrange("p (h q) -> p h q", h=2)
                    )

                def av_pair(k0, ptile, q0a, start, stop):
                    for e, o in ((0, oa), (1, ob)):
                        nc.tensor.matmul(
                            o[:, q0a:512],
                            v2e[p][:, k0 : k0 + 2, 128 * e : 128 * e + 128],
                            ptile[:, :, e * 512 + q0a : (e + 1) * 512],
                            start=start,
                            stop=stop,
                            perf_mode=DR,
                        )

                # units: DoubleRow pairs; non-crossing pairs full-width, the
                # two crossing pairs trimmed to q0a with [zero|tri] masking on
                # the odd member.  AV issue lags scores by 2 units so the PE
                # never stalls on the exp/mask chain.  Some non-crossing exps
                # run as Schraudolph fast-exp on DVE to unload ScalarE.
                units = [("nc", j) for j in range(2 * qb)]
                units += [("cr", r) for r in range(2)]
                tiles = {}
                pending = []
                nunit = len(units)

                def issue_av(idx):
                    kind, a = units[idx]
                    start = idx == 0
                    stop = idx == nunit - 1
                    if kind == "nc":
                        av_pair(2 * a, tiles[idx], 0, start, stop)
                    else:
                        av_pair(4 * qb + 2 * a, tiles[idx], 256 * a, start, stop)

                for idx, (kind, a) in enumerate(units):
                    ptile = pc.tile([128, 2, 1024], F8, tag="pt", name="pt", bufs=6)
                    tiles[idx] = ptile
                    if kind == "nc":
                        offload = qb >= 2 and a in SCHRAUD.get(qb, ())
                        for half in range(2):
                            s2 = scores(2 * a + half, 0)
                            if offload:
                                exp_schraudolph(s2, ptile, half)
                            else:
                                exp_full(s2, ptile, half)
                    else:
                        q0a = 256 * a
                        for half in range(2):
                            s2 = scores(4 * qb + 2 * a + half, q0a)
                            exp_strided(s2, ptile, half, q0a)
                            mask_strip(ptile, half, q0a, zt=(half == 1))
                    pending.append(idx)
                    if filler is not None:
                        next(filler, None)
                    if len(pending) > 3:
                        issue_av(pending.pop(0))
                for idx in pending:
                    issue_av(idx)
                if filler is not None:
                    for _ in filler:
                        pass

                stage = pc.tile([65, 2, 512], F16, tag="stage", name="stage")
                nc.vector.tensor_copy(stage[:, 0, :], oa[0:65, :])
                nc.vector.tensor_copy(stage[:, 1, :], ob[0:65, :])
                nc.sync.dma_start(
                    out=out.rearrange("h (nl q) -> h nl q", nl=HPC)[
                        :, 2 * p : 2 * p + 2, qb * 512 : (qb + 1) * 512
                    ],
                    in_=stage[:],
                )

            # interleave: the second projection chunk's steps are spread
            # between the early attention blocks so projection matmuls fill
            # the PE while ScalarE drains the attention exps (and vice versa)
            for p in range(NPAIR):
                for t in range(3):
                    proj_step(0, p, t)
            attention(0, 1)
            proj_step(1, 0, 0)
            attention(1, 1)
            proj_step(1, 0, 1)
            attention(0, 0)
            proj_step(1, 0, 2)
            attention(1, 0)
            proj_step(1, 1, 0)
            proj_step(1, 1, 1)
            proj_step(1, 1, 2)
            attention(0, 3)
            attention(1, 3)
            attention(0, 2)
            attention(1, 2)

    if split_waits:
        _split_excess_waits(nc)
    return nc


_NC_CACHE = None


def _get_nc():
    global _NC_CACHE
    if _NC_CACHE is None:
        _NC_CACHE = build_nc()
    return _NC_CACHE


def _host_inputs(x, W_qkv):
    """Per-core input maps."""
    xT8 = [np.ascontiguousarray(x[b].T).astype(NP8) for b in range(B)]
    # w[pair, t, dtile, 128, 128]: cols 0:64 head a, 64:128 head b
    Wr = np.ascontiguousarray((W_qkv * WSCALE).reshape(NH, DT, 128, 3, HD))
    ki = np.arange(128)[:, None]
    qj = np.arange(128)[None, :]
    tri1 = (ki <= qj).astype(np.float32)
    tri = np.concatenate([tri1, tri1], axis=1).astype(NP8)  # [128, 256]
    z1 = np.zeros((128, 128), dtype=np.float32)
    trizt = np.concatenate([z1, tri1, z1, tri1], axis=1).astype(NP8)  # [128, 512]
    ident = np.eye(128, dtype=np.float16)
    in_maps = []
    for c in range(NCORES):
        b = c // 4
        h0 = 4 * (c % 4)
        w = np.empty((NPAIR, 3, DT, 128, 128), dtype=np.float32)
        for p in range(NPAIR):
            ha, hb = h0 + 2 * p, h0 + 2 * p + 1
            for t in range(3):
                w[p, t, :, :, 0:64] = Wr[ha, :, :, t, :]
                w[p, t, :, :, 64:128] = Wr[hb, :, :, t, :]
        in_maps.append(
            {"xT": xT8[b], "w": w.astype(NP8), "tri": tri, "trizt": trizt, "ident": ident}
        )
    return in_maps


def _patch_early_rows(O, x, W_qkv):
    """Exact recompute of rows < PATCH_ROWS: with tiny softmax support the
    fp8 V/P quantization error doesn't average out there."""
    R = PATCH_ROWS
    if not R:
        return
    xr = x[:, :R].astype(np.float64)  # [B, R, D]
    W = W_qkv.astype(np.float64)
    qkv = np.einsum("brd,ndh->bnrh", xr, W)  # [B, N, R, 3H]
    q, k, v = np.split(qkv, 3, axis=-1)
    sc = np.einsum("bnrh,bnsh->bnrs", q, k) / np.sqrt(HD)
    tril = np.tril(np.ones((R, R)))
    P = np.exp(sc) * tril[None, None]
    o = P @ v / P.sum(-1)[..., None]  # [B, N, R, H]
    O[:, :, :R, :] = o.astype(np.float32)


def _host_epilogue(results, x, W_qkv, W_out):
    # V was computed from W_v*16, so the numerator rows carry a 16x factor.
    W_sum = W_out.sum(axis=(0, 1)).astype(np.float32)  # [D]
    O = np.empty((B, NH, S, HD), dtype=np.float32)
    for c in range(NCORES):
        o = results[c]["out"].astype(np.float32)  # [65, 4*2048]
        b = c // 4
        h0 = 4 * (c % 4)
        body = o[0:64].reshape(64, HPC, S)  # [h, nl, s]
        den = o[64].reshape(HPC, S)  # [nl, s]
        O[b, h0 : h0 + HPC] = body.transpose(1, 2, 0) / (WSCALE * den[:, :, None])
    _patch_early_rows(O, x, W_qkv)
    out2 = O.reshape(B, D, S)  # raw row-major reshape, as in the reference
    return np.ascontiguousarray(
        out2.transpose(0, 2, 1) * W_sum[None, None, :]
    ).astype(np.float32)


def _run(x, W_qkv, W_out, trace=False):
    nc = _get_nc()
    in_maps = _host_inputs(x, W_qkv)
    res = run_bass_kernel_spmd(
        nc,
        in_maps,
        list(range(NCORES)),
        trace=trace,
        trace_cores=list(range(NCORES)) if trace else None,
    )
    return _host_epilogue(res.results, x, W_qkv, W_out), res


def kernel(x, W_qkv, W_out):
    x = np.asarray(x, dtype=np.float32)
    W_qkv = np.asarray(W_qkv, dtype=np.float32)
    W_out = np.asarray(W_out, dtype=np.float32)
    out, _ = _run(x, W_qkv, W_out, trace=False)
    return out


def kernel_traced(x, W_qkv, W_out):
    out, res = _run(
        np.asarray(x, np.float32),
        np.asarray(W_qkv, np.float32),
        np.asarray(W_out, np.float32),
        trace=True,
    )
    return out, res
